# revision 1
# baseline (speedup 1.0000x reference)
"""Trainium2 Bass kernel for attribute visual attention.

Computes, for each batch b:
    q      = v @ W_alpha                  # [i, f]
    scores = q @ vf[b]                    # [i, r]
    atten  = softmax(scores, axis=r)
    out[b] = atten @ vf[b].T              # [i, f]

Sharding: data-parallel over batch b across 8 NeuronCores (8 batches per
core); v / W_alpha replicated. All matmuls run in fp16 (full PE rate on
TRN2) with fp32 PSUM accumulation; softmax statistics in fp32.

Layout notes:
- The attend matmul contracts over r, which must live on SBUF partitions
  for both operands; the host passes visual_features twice — [f, r] for
  the scores matmul and pre-transposed [r, f] for the attend matmul. The
  small e = exp(scores - max) matrix is transposed on-chip with the DMA
  xbar (fp16).
- Batches are processed in PAIRS for the scores matmul (rhs = two
  batches side by side, N=392): halves the number of PE instructions and
  stationary-weight loads.
- Bulk HBM traffic uses SWDGE (gpsimd) so the shared HWDGE block is left
  for the xbar transposes.
- Softmax normalization is folded into the PSUM->SBUF output copy as a
  per-partition scale.
"""

import numpy as np
from contextlib import ExitStack

import concourse.bass as bass
import concourse.tile as tile
import concourse.bass_utils as bass_utils
from concourse import bacc, mybir

# Problem shapes (hardcoded per contest contract).
B, F, R, I, V = 64, 2048, 196, 312, 300
NCORES = 8
BL = B // NCORES          # 8 batches per core
NPAIR = BL // 2           # 4 batch-pairs per core
FT = F // 128             # 16 f-tiles
RPAD = 256                # r padded to 2x128 for the xbar transpose
I_TILES = ((0, 128), (128, 128), (256, 56))
KV_TILES = ((0, 128), (128, 128), (256, 44))    # v=300
KR_TILES = ((0, 128), (128, 68))                # r=196

F16 = mybir.dt.float16
F32 = mybir.dt.float32

_CACHE = {}


def _build_body(nc, tc, ctx, wa, vt, vf, vft, ident, out, reps):
    qtp = ctx.enter_context(tc.tile_pool(name="qt", bufs=1))
    ident_t = qtp.tile([128, 128], F16, tag="ident", name="ident")
    with tc.high_priority():
        nc.sync.dma_start(ident_t[:], ident[:])

    # PE warm-up: ~30 junk matmuls on the identity while the weight loads are
    # still in flight, so the clock ramp completes before real work starts
    with tc.tile_pool(name="wupsum", bufs=1, space=bass.MemorySpace.PSUM) as wup:
        wu = wup.tile([128, 128], F32, tag="wu", name="wu")
        for w in range(55):
            nc.tensor.matmul(wu[:], ident_t[:], ident_t[:],
                             start=(w == 0), stop=(w == 54))

    # ---- Phase 0: qT[f, i] = (v @ W_alpha).T via lhsT=W_alpha, rhs=v.T ----
    qt_t = []
    with tc.tile_pool(name="const", bufs=1) as const, \
         tc.tile_pool(name="qpsum", bufs=2, space=bass.MemorySpace.PSUM) as qpsum:
        wa_t, vt_t = [], []
        for k, (v0, vs) in enumerate(KV_TILES):
            t = const.tile([vs, I], F16, tag=f"vt{k}")
            with tc.high_priority():
                nc.sync.dma_start(t[:], vt[v0:v0 + vs, :])
            vt_t.append(t)
        for k, (v0, vs) in enumerate(KV_TILES):
            w = const.tile([vs, F], F16, tag=f"wa{k}")
            with tc.high_priority():
                for c in range(2):
                    nc.sync.dma_start(w[:, c * 1024:(c + 1) * 1024],
                                      wa[v0:v0 + vs, c * 1024:(c + 1) * 1024])
            wa_t.append(w)

        for mf in range(FT):
            qp = qpsum.tile([128, I], F32, tag="qp")
            for k, (v0, vs) in enumerate(KV_TILES):
                nc.tensor.matmul(qp[:], wa_t[k][:, mf * 128:(mf + 1) * 128],
                                 vt_t[k][:], start=(k == 0), stop=(k == 2))
            q = qtp.tile([128, I], F16, tag=f"qt{mf}")
            nc.scalar.copy(q[:], qp[:])
            qt_t.append(q)

    # ---- Phase 1: per batch-pair attention ----
    vfp = ctx.enter_context(tc.tile_pool(name="vf", bufs=4))
    vftp = ctx.enter_context(tc.tile_pool(name="vft", bufs=4))
    esp = ctx.enter_context(tc.tile_pool(name="es", bufs=6))
    attp = ctx.enter_context(tc.tile_pool(name="atT", bufs=3))
    outp = ctx.enter_context(tc.tile_pool(name="out", bufs=2))
    stat = ctx.enter_context(tc.tile_pool(name="stat", bufs=8))
    spsum = ctx.enter_context(
        tc.tile_pool(name="spsum", bufs=3, space=bass.MemorySpace.PSUM))
    opsum = ctx.enter_context(
        tc.tile_pool(name="opsum", bufs=4, space=bass.MemorySpace.PSUM))
    tpsum = ctx.enter_context(
        tc.tile_pool(name="tpsum", bufs=1, space=bass.MemorySpace.PSUM))

    PW = 1     # pairs per wave
    for rep in range(reps):
        for half in range(NPAIR // PW):
            if half > 0:
                # PSUM-free PE activity across the DMA-bound wave boundary:
                # standalone weight loads keep the clock-ramp monitor fed
                for _ in range(10):
                    nc.tensor.ldweights(ident_t[:])
            # vf pair tiles: [128, t, j*196+r] for the wave's batch pairs
            vf_t, vft_t = [], {}
            for p in range(PW):
                bp = half * PW + p
                vt_ = vfp.tile([128, FT, 2 * R], F16, tag="vf", name=f"vf{p}")
                with tc.high_priority():
                    for c in range(4):
                        nc.gpsimd.dma_start(vt_[:, 4 * c:4 * (c + 1), :],
                                            vf[bp, :, 4 * c:4 * (c + 1), :])
                vf_t.append(vt_)
                for j in range(2):
                    b = 2 * bp + j
                    jj = 2 * p + j
                    for kr, (r0, rs) in enumerate(KR_TILES):
                        vv = vftp.tile([rs, F], F16, tag=f"vft{kr}{jj}",
                                       name=f"vft{kr}{jj}")
                        with tc.high_priority():
                            nc.gpsimd.dma_start(vv[:], vft[b, r0:r0 + rs, :])
                        vft_t[(jj, kr)] = vv

            esT_full = [
                [attp.tile([rs, I], F16, tag=f"esT{kr}{jj % 2}",
                           name=f"esT{kr}{jj % 2}")
                 for kr, (r0, rs) in enumerate(KR_TILES)]
                for jj in range(2 * PW)]
            for mi, (i0, isz) in enumerate(I_TILES):
                # scores for all wave batches; inner loop over pairs so the
                # stationary qT tile is reused PW times per load
                sps = [spsum.tile([isz, 2, R], F32, tag="sp", name=f"sp{p}")
                       for p in range(PW)]
                for kf in range(FT):
                    for p in range(PW):
                        nc.tensor.matmul(
                            sps[p][:], qt_t[kf][:, i0:i0 + isz],
                            vf_t[p][:, kf, :].rearrange("p (j r) -> p j r", j=2),
                            start=(kf == 0), stop=(kf == FT - 1))

                for p in range(PW):
                    sp = sps[p]
                    negmax = stat.tile([isz, 2], F32, tag="negmax")
                    with tc.high_priority():
                        nc.vector.tensor_reduce(negmax[:], sp[:],
                                                axis=mybir.AxisListType.X,
                                                op=mybir.AluOpType.max, negate=True)
                    sums = stat.tile([isz, 2], F32, tag="sums")
                    rcp = stat.tile([isz, 2], F32, tag="rcp")
                    for j in range(2):
                        jj = 2 * p + j
                        es = esp.tile([128, R], F16, tag="es")
                        att = esp.tile([128, R], F16, tag="att")
                        with tc.high_priority():
                            nc.scalar.activation(es[:isz, 0:R], sp[:, j, :],
                                                 mybir.ActivationFunctionType.Exp,
                                                 bias=negmax[:, j:j + 1],
                                                 scale=1.0,
                                                 accum_out=sums[:, j:j + 1])
                            nc.vector.reciprocal(rcp[:, j:j + 1],
                                                 sums[:, j:j + 1])
                            # normalize while atten is still i-partitioned
                            nc.vector.tensor_scalar_mul(att[:isz, :],
                                                        es[:isz, :],
                                                        rcp[:, j:j + 1])

                        # transpose atten -> attenT[r, i-slice] on the PE
                        # (transpose-mode matmul against identity); accumulate
                        # the full [r, 312] attenT in SBUF across i-tiles
                        for kr, (r0, rs) in enumerate(KR_TILES):
                            tp = tpsum.tile([rs, isz], F16, tag="tp",
                                            name=f"tp{kr}")
                            with tc.high_priority():
                                nc.tensor.transpose(tp[:], att[:isz, r0:r0 + rs],
                                                    ident_t[0:isz, 0:isz])
                                nc.vector.tensor_copy(
                                    esT_full[jj][kr][:, i0:i0 + isz], tp[:])

            # attend (transposed output): outT[f, i] = vfT.T @ attenT,
            # M=f (16 exact tiles), N=i=312 -- no tile waste
            for jj in range(2 * PW):
                b = 2 * half * PW + jj
                otf = outp.tile([128, FT, I], F16, tag=f"otf{jj % 2}",
                                name=f"otf{jj % 2}")
                for mf in range(FT):
                    op_ = opsum.tile([128, I], F32, tag="op", name="op")
                    for kr, (r0, rs) in enumerate(KR_TILES):
                        nc.tensor.matmul(
                            op_[:], vft_t[(jj, kr)][:, mf * 128:(mf + 1) * 128],
                            esT_full[jj][kr][:],
                            start=(kr == 0), stop=(kr == 1))
                    if mf % 2 == 0:
                        nc.scalar.copy(otf[:, mf, :], op_[:])
                    else:
                        nc.vector.tensor_copy(otf[:, mf, :], op_[:])
                for c in range(4):
                    nc.sync.dma_start(out[b, :, 4 * c:4 * (c + 1), :],
                                      otf[:, 4 * c:4 * (c + 1), :])


def _get_program(reps=1):
    key = ("nc", reps)
    if key in _CACHE:
        return _CACHE[key]
    nc = bacc.Bacc("TRN2", target_bir_lowering=False, debug=False,
                   num_devices=NCORES)
    wa_d = nc.dram_tensor("walpha", [V, F], F16, kind="ExternalInput")
    vt_d = nc.dram_tensor("vt", [V, I], F16, kind="ExternalInput")
    vf_d = nc.dram_tensor("vf", [NPAIR, 128, FT, 2 * R], F16,
                          kind="ExternalInput")
    vft_d = nc.dram_tensor("vft", [BL, R, F], F16, kind="ExternalInput")
    id_d = nc.dram_tensor("ident", [128, 128], F16, kind="ExternalInput")
    out_d = nc.dram_tensor("out", [BL, 128, FT, I], F16,
                           kind="ExternalOutput")

    with tile.TileContext(nc) as tc, ExitStack() as ctx:
        _build_body(nc, tc, ctx, wa_d.ap(), vt_d.ap(), vf_d.ap(),
                    vft_d.ap(), id_d.ap(), out_d.ap(), reps)
    nc.compile()
    _CACHE[key] = nc
    return nc


def _prep_inputs(visual_features, v, W_alpha):
    vf = np.asarray(visual_features, dtype=np.float32)
    v = np.asarray(v, dtype=np.float32)
    W = np.asarray(W_alpha, dtype=np.float32)

    walpha16 = np.ascontiguousarray(W).astype(np.float16)          # [V, F]
    vt16 = np.ascontiguousarray(v.T).astype(np.float16)            # [V, I]
    # [b, f, r] -> [bp, p=128, t=16, j*196+r]: batch-paired, per-partition
    # contiguous DMA layout
    vf16 = np.ascontiguousarray(
        vf.reshape(B // 2, 2, FT, 128, R).transpose(0, 3, 2, 1, 4)
        .reshape(B // 2, 128, FT, 2 * R)).astype(np.float16)
    vft16 = np.ascontiguousarray(vf.transpose(0, 2, 1)).astype(np.float16)

    in_maps = []
    for c in range(NCORES):
        in_maps.append({
            "walpha": walpha16,
            "vt": vt16,
            "ident": np.eye(128, dtype=np.float16),
            "vf": np.ascontiguousarray(vf16[c * NPAIR:(c + 1) * NPAIR]),
            "vft": np.ascontiguousarray(vft16[c * BL:(c + 1) * BL]),
        })
    return in_maps


def kernel(visual_features, v, W_alpha):
    nc = _get_program()
    in_maps = _prep_inputs(visual_features, v, W_alpha)
    res = None
    for attempt in range(3):
        try:
            res = bass_utils.run_bass_kernel_spmd(
                nc, in_maps, core_ids=list(range(NCORES)))
            break
        except Exception:
            # transient NRT_EXEC_UNIT_UNRECOVERABLE wedges have been seen on
            # this fabric; a re-dispatch typically succeeds
            if attempt == 2:
                raise
    outs = [res.results[c]["out"] for c in range(NCORES)]
    buf = np.concatenate(outs, axis=0)          # [B, p=128, t=16, I]
    full = buf.transpose(0, 3, 2, 1).reshape(B, I, F)   # f = t*128 + p
    return np.ascontiguousarray(full).astype(np.float32)



# revision 5
# speedup vs baseline: 1.0366x; 1.0366x over previous
"""Trainium2 Bass kernel for attribute visual attention.

Computes, for each batch b:
    q      = v @ W_alpha                  # [i, f]
    scores = q @ vf[b]                    # [i, r]
    atten  = softmax(scores, axis=r)
    out[b] = atten @ vf[b].T              # [i, f]

Sharding: data-parallel over batch b across 8 NeuronCores (8 batches per
core); v / W_alpha replicated.

Design (v2 — transposed-scores pipeline):
- scores are computed TRANSPOSED: scoresT[r, i] = vf[b].T-contract via
  lhsT = vf[b] in its natural [f, r] layout, rhs = qT[f, i]. The softmax
  then needs no on-chip transposes at all: exp runs on scoresT directly
  (partition dim = r), and the attend matmul consumes esT[r, i] as the
  moving operand with lhsT = vfT[b] slices.
- No per-row max subtraction. scores*1.0 - 30.0 feeds Exp; results are
  stored in bf16 (range to 3e38 absorbs exp(~86) tails, and the +/-30
  shift cancels in the normalization). The softmax denominator comes from
  a ones-matmul that broadcasts the per-column sum to all 128 partitions
  in PSUM for free; normalization is a single DVE multiply on the small
  esT tiles (atten stored fp16 once normalized, values <= 1).
- PE work is software-pipelined as scoresT(k) -> attend(k-1) -> sums(k)
  so the tensor engine never waits on the ACT/DVE softmax chain.
- All inputs are DMA'd up front (everything fits in SBUF); inputs ride
  the HWDGE (sync) queue, outputs the SWDGE (gpsimd) queue.
- Attend outputs drain from PSUM via dual-bank copies (two 312-col tiles
  per instruction, alternating scalar/vector engines).
"""

import numpy as np
from contextlib import ExitStack

import concourse.bass as bass
import concourse.tile as tile
import concourse.bass_utils as bass_utils
from concourse import bacc, mybir

# Problem shapes (hardcoded per contest contract).
B, F, R, I, V = 64, 2048, 196, 312, 300
NCORES = 8
BL = B // NCORES          # 8 batches per core
FT = F // 128             # 16 f-tiles
KV_TILES = ((0, 128), (128, 128), (256, 44))    # v=300
KR_TILES = ((0, 128), (128, 68))                # r=196
EXP_SHIFT = -30.0

F16 = mybir.dt.float16
BF16 = mybir.dt.bfloat16
F32 = mybir.dt.float32

_CACHE = {}


def _build_body(nc, tc, ctx, wa, vt, vfp, vft, out, reps):
    constp = ctx.enter_context(tc.tile_pool(name="const", bufs=1))
    vfpp = ctx.enter_context(tc.tile_pool(name="vfp", bufs=1))
    vftp = ctx.enter_context(tc.tile_pool(name="vft", bufs=1))

    junk = constp.tile([128, 128], F16, tag="junk")
    ones = constp.tile([128, 128], BF16, tag="ones")
    ebias = constp.tile([128, 1], F32, tag="ebias")
    nc.gpsimd.memset(junk[:], 0.03125)
    nc.gpsimd.memset(ones[:], 1.0)
    nc.gpsimd.memset(ebias[:], EXP_SHIFT)

    # ---- all input DMAs up front (everything is SBUF-resident) ----
    vt_t, wa_t = [], []
    with tc.high_priority():
        for k, (v0, vs) in enumerate(KV_TILES):
            t = constp.tile([vs, I], F16, tag=f"vt{k}")
            nc.sync.dma_start(t[:], vt[v0:v0 + vs, :])
            vt_t.append(t)
        for c in range(4):
            for k, (v0, vs) in enumerate(KV_TILES):
                w = constp.tile([vs, 512], F16, tag=f"wa{k}_{c}")
                nc.sync.dma_start(w[:], wa[v0:v0 + vs, c * 512:(c + 1) * 512])
                wa_t.append((c, k, w))
    vfp_t, vft_t = [], {}
    for b in range(BL):
        t = vfpp.tile([128, FT, R], F16, tag=f"vfp{b}")
        nc.sync.dma_start(t[:], vfp[b])
        vfp_t.append(t)
        for kr, (r0, rs) in enumerate(KR_TILES):
            v = vftp.tile([rs, F], F16, tag=f"vft{b}_{kr}")
            nc.sync.dma_start(v[:], vft[b, r0:r0 + rs, :])
            vft_t[(b, kr)] = v

    # ---- PE warm-up while the first weight chunks land ----
    with tc.tile_pool(name="wupsum", bufs=1, space=bass.MemorySpace.PSUM) as wup:
        wu = wup.tile([128, 128], F32, tag="wu")
        for w in range(34):
            nc.tensor.matmul(wu[:], junk[:], junk[:],
                             start=(w == 0), stop=(w == 33))

    # ---- Phase 0: qT[f, i] tiles: lhsT=W_alpha chunk, rhs=v.T ----
    qt_t = []
    with tc.tile_pool(name="qpsum", bufs=2, space=bass.MemorySpace.PSUM) as qpsum:
        for mf in range(FT):
            c, sub = mf // 4, mf % 4
            qp = qpsum.tile([128, I], F32, tag="qp")
            for k, (v0, vs) in enumerate(KV_TILES):
                w = wa_t[c * 3 + k][2]
                nc.tensor.matmul(qp[:], w[:, sub * 128:(sub + 1) * 128],
                                 vt_t[k][:], start=(k == 0), stop=(k == 2))
            q = constp.tile([128, I], F16, tag=f"qt{mf}")
            nc.scalar.copy(q[:], qp[:])
            qt_t.append(q)

    # ---- Phase 1: per-batch attention, PE-pipelined ----
    esp = ctx.enter_context(tc.tile_pool(name="es", bufs=4))
    attp = ctx.enter_context(tc.tile_pool(name="att", bufs=4))
    rcpp = ctx.enter_context(tc.tile_pool(name="rcp", bufs=2))
    outp = ctx.enter_context(tc.tile_pool(name="out", bufs=2))
    spsum = ctx.enter_context(
        tc.tile_pool(name="spsum", bufs=1, space=bass.MemorySpace.PSUM))
    smpsum = ctx.enter_context(
        tc.tile_pool(name="smpsum", bufs=2, space=bass.MemorySpace.PSUM))
    opsum = ctx.enter_context(
        tc.tile_pool(name="opsum", bufs=2, space=bass.MemorySpace.PSUM))

    def scores_exp(b):
        es_t = []
        for kr, (r0, rs) in enumerate(KR_TILES):
            sp = spsum.tile([rs, I], F32, tag=f"sp{kr}", name=f"sp{kr}")
            for kf in range(FT):
                nc.tensor.matmul(sp[:], vfp_t[b][:, kf, r0:r0 + rs],
                                 qt_t[kf][:], start=(kf == 0),
                                 stop=(kf == FT - 1))
            es = esp.tile([rs, I], BF16, tag=f"es{kr}", name=f"es{kr}")
            with tc.high_priority():
                nc.scalar.activation(es[:], sp[:],
                                     mybir.ActivationFunctionType.Exp,
                                     bias=ebias[0:rs, :], scale=1.0)
            es_t.append(es)
        return es_t

    def sums_rcp_norm(b, es_t):
        sm = smpsum.tile([128, I], F32, tag="sm", name="sm")
        for kr, (r0, rs) in enumerate(KR_TILES):
            nc.tensor.matmul(sm[:], ones[0:rs, :], es_t[kr][:],
                             start=(kr == 0), stop=(kr == 1))
        rcpb = rcpp.tile([128, I], F32, tag="rcpb", name="rcpb")
        att_t = []
        with tc.high_priority():
            nc.vector.reciprocal(rcpb[:], sm[:])
            for kr, (r0, rs) in enumerate(KR_TILES):
                at = attp.tile([rs, I], F16, tag=f"at{kr}", name=f"at{kr}")
                nc.vector.tensor_tensor(at[:], es_t[kr][:], rcpb[0:rs, :],
                                        mybir.AluOpType.mult)
                att_t.append(at)
        return att_t

    def attend(b, att_t):
        otf = outp.tile([128, FT, I], F16, tag="otf", name="otf")
        for md in range(FT // 2):
            op_ = opsum.tile([128, 2, 512], F32, tag="op", name="op")
            for j in range(2):
                mf = 2 * md + j
                for kr, (r0, rs) in enumerate(KR_TILES):
                    nc.tensor.matmul(
                        op_[:, j, 0:I],
                        vft_t[(b, kr)][:, mf * 128:(mf + 1) * 128],
                        att_t[kr][:], start=(kr == 0), stop=(kr == 1))
            dst = otf[:, 2 * md:2 * md + 2, :]
            src = op_[:, :, 0:I]
            if md % 2 == 0:
                nc.scalar.copy(dst, src)
            else:
                nc.vector.tensor_copy(dst, src)
        for c in range(2):
            nc.gpsimd.dma_start(out[b, :, 8 * c:8 * (c + 1), :],
                                otf[:, 8 * c:8 * (c + 1), :])

    for rep in range(reps):
        prev = None   # (b, att_t)
        for b in range(BL):
            es_t = scores_exp(b)
            if prev is not None:
                attend(*prev)
            att_t = sums_rcp_norm(b, es_t)
            prev = (b, att_t)
        attend(*prev)


def _get_program(reps=1):
    key = ("nc", reps)
    if key in _CACHE:
        return _CACHE[key]
    nc = bacc.Bacc("TRN2", target_bir_lowering=False, debug=False,
                   num_devices=NCORES)
    wa_d = nc.dram_tensor("walpha", [V, F], F16, kind="ExternalInput")
    vt_d = nc.dram_tensor("vt", [V, I], F16, kind="ExternalInput")
    vfp_d = nc.dram_tensor("vfp", [BL, 128, FT, R], F16, kind="ExternalInput")
    vft_d = nc.dram_tensor("vft", [BL, R, F], F16, kind="ExternalInput")
    out_d = nc.dram_tensor("out", [BL, 128, FT, I], F16,
                           kind="ExternalOutput")

    with tile.TileContext(nc) as tc, ExitStack() as ctx:
        _build_body(nc, tc, ctx, wa_d.ap(), vt_d.ap(), vfp_d.ap(),
                    vft_d.ap(), out_d.ap(), reps)
    nc.compile()
    _CACHE[key] = nc
    return nc


def _prep_inputs(visual_features, v, W_alpha):
    vf = np.asarray(visual_features, dtype=np.float32)
    v = np.asarray(v, dtype=np.float32)
    W = np.asarray(W_alpha, dtype=np.float32)

    walpha16 = np.ascontiguousarray(W).astype(np.float16)          # [V, F]
    vt16 = np.ascontiguousarray(v.T).astype(np.float16)            # [V, I]
    # [b, f, r] -> [b, p=128, t=16, r]  (f = t*128 + p)
    vfp16 = np.ascontiguousarray(
        vf.reshape(B, FT, 128, R).transpose(0, 2, 1, 3)).astype(np.float16)
    vft16 = np.ascontiguousarray(vf.transpose(0, 2, 1)).astype(np.float16)

    in_maps = []
    for c in range(NCORES):
        in_maps.append({
            "walpha": walpha16,
            "vt": vt16,
            "vfp": np.ascontiguousarray(vfp16[c * BL:(c + 1) * BL]),
            "vft": np.ascontiguousarray(vft16[c * BL:(c + 1) * BL]),
        })
    return in_maps


def kernel(visual_features, v, W_alpha):
    nc = _get_program()
    in_maps = _prep_inputs(visual_features, v, W_alpha)
    res = None
    for attempt in range(3):
        try:
            res = bass_utils.run_bass_kernel_spmd(
                nc, in_maps, core_ids=list(range(NCORES)))
            break
        except Exception:
            # transient NRT_EXEC_UNIT_UNRECOVERABLE wedges have been seen on
            # this fabric; a re-dispatch typically succeeds
            if attempt == 2:
                raise
    outs = [res.results[c]["out"] for c in range(NCORES)]
    buf = np.concatenate(outs, axis=0)          # [B, p=128, t=16, I]
    full = buf.transpose(0, 3, 2, 1).reshape(B, I, F)   # f = t*128 + p
    return np.ascontiguousarray(full).astype(np.float32)


# revision 28
# speedup vs baseline: 1.2459x; 1.2019x over previous
"""Trainium2 Bass kernel for attribute visual attention.

Computes, for each batch b:
    q      = v @ W_alpha                  # [i, f]
    scores = q @ vf[b]                    # [i, r]
    atten  = softmax(scores, axis=r)
    out[b] = atten @ vf[b].T              # [i, f]

Sharding: data-parallel over batch b across 8 NeuronCores (8 batches per
core); v / W_alpha replicated.

Design (v2 — transposed-scores pipeline):
- scores are computed TRANSPOSED: scoresT[r, i] = vf[b].T-contract via
  lhsT = vf[b] in its natural [f, r] layout, rhs = qT[f, i]. The softmax
  then needs no on-chip transposes at all: exp runs on scoresT directly
  (partition dim = r), and the attend matmul consumes esT[r, i] as the
  moving operand with lhsT = vfT[b] slices.
- No per-row max subtraction. scores*1.0 - 30.0 feeds Exp; results are
  stored in bf16 (range to 3e38 absorbs exp(~86) tails, and the +/-30
  shift cancels in the normalization). The softmax denominator comes from
  a ones-matmul that broadcasts the per-column sum to all 128 partitions
  in PSUM for free; normalization is a single DVE multiply on the small
  esT tiles (atten stored fp16 once normalized, values <= 1).
- PE work is software-pipelined as scoresT(k) -> attend(k-1) -> sums(k)
  so the tensor engine never waits on the ACT/DVE softmax chain.
- All inputs are DMA'd up front (everything fits in SBUF); inputs ride
  the HWDGE (sync) queue, outputs the SWDGE (gpsimd) queue.
- Attend outputs drain from PSUM via dual-bank copies (two 312-col tiles
  per instruction, alternating scalar/vector engines).
"""

import numpy as np
from contextlib import ExitStack

import concourse.bass as bass
import concourse.tile as tile
import concourse.bass_utils as bass_utils
from concourse import bacc, mybir

# Problem shapes (hardcoded per contest contract).
B, F, R, I, V = 64, 2048, 196, 312, 300
NCORES = 8
BL = B // NCORES          # 8 batches per core
FT = F // 128             # 16 f-tiles
KV_TILES = ((0, 128), (128, 128), (256, 44))    # v=300
KR_TILES = ((0, 128), (128, 68))                # r=196
EXP_SHIFT = -30.0

F16 = mybir.dt.float16
BF16 = mybir.dt.bfloat16
F32 = mybir.dt.float32

_CACHE = {}


WQ = I + F     # waq packed width: [vT | W_alpha]
WH = WQ // 2   # half-column DMA chunk


def _build_body(nc, tc, ctx, waq, vfp, vft, out, reps):
    constp = ctx.enter_context(tc.tile_pool(name="const", bufs=1))
    vfpp = ctx.enter_context(tc.tile_pool(name="vfp", bufs=1))
    vftp = ctx.enter_context(tc.tile_pool(name="vft", bufs=1))

    junk = constp.tile([128, 128], F16, tag="junk")
    ones = constp.tile([128, 128], BF16, tag="ones")
    ebias = constp.tile([128, 1], F32, tag="ebias")
    nc.vector.memset(junk[:], 0.03125)
    nc.vector.memset(ones[:], 1.0)
    nc.vector.memset(ebias[:], EXP_SHIFT)

    # ---- all input DMAs up front (everything is SBUF-resident).
    # Weights ride SP's HWDGE queue; the bulk vf loads ride the Pool/SWDGE
    # queue whose descriptor generator then has nothing else to do, so the
    # per-batch supply stays ahead of per-batch demand.  Outputs use SP. ----
    wa_t = {}
    with tc.high_priority():
        for h in range(2):
            for k, (v0, vs) in enumerate(KV_TILES):
                if h == 0:
                    w = constp.tile([vs, WQ], F16, tag=f"wa{k}", name=f"wa{k}")
                    wa_t[k] = w
                nc.sync.dma_start(wa_t[k][:, h * WH:(h + 1) * WH],
                                  waq[v0:v0 + vs, h * WH:(h + 1) * WH])
    vfp_t, vft_t = [], {}
    for b in range(BL):
        t = vfpp.tile([128, FT, R], F16, tag=f"vfp{b}")
        # early batches in t-chunks so the weight DMAs aren't stuck behind a
        # monolithic transfer and the fused batch-0 scores can start early
        nch = 4 if b == 0 else (2 if b == 1 else 1)
        step = FT // nch
        for c in range(nch):
            nc.gpsimd.dma_start(t[:, c * step:(c + 1) * step, :],
                                vfp[b, :, c * step:(c + 1) * step, :])
        vfp_t.append(t)
        for kr, (r0, rs) in enumerate(KR_TILES):
            v = vftp.tile([rs, F], F16, tag=f"vft{b}_{kr}")
            nc.gpsimd.dma_start(v[:], vft[b, r0:r0 + rs, :])
            vft_t[(b, kr)] = v

    # ---- PE warm-up while the first weight chunks land ----
    with tc.tile_pool(name="wupsum", bufs=1, space=bass.MemorySpace.PSUM) as wup:
        wu = wup.tile([128, 128], F32, tag="wu")
        for w in range(32):
            nc.tensor.matmul(wu[:], junk[:], junk[:],
                             start=(w == 0), stop=(w == 31))

    # ---- Phase 1: per-batch attention, PE-pipelined ----
    esp = ctx.enter_context(tc.tile_pool(name="es", bufs=4))
    attp = ctx.enter_context(tc.tile_pool(name="att", bufs=4))
    rcpp = ctx.enter_context(tc.tile_pool(name="rcp", bufs=2))
    outp = ctx.enter_context(tc.tile_pool(name="out", bufs=2))
    spsum = ctx.enter_context(
        tc.tile_pool(name="spsum", bufs=1, space=bass.MemorySpace.PSUM))
    smpsum = ctx.enter_context(
        tc.tile_pool(name="smpsum", bufs=1, space=bass.MemorySpace.PSUM))

    # ---- Phase 0 fused with batch 0's scores: the qT tiles are produced
    # two steps ahead of their use by scoresT(0), so batch 0's scores finish
    # ~right after the last q tile instead of a full scores-pass later. ----
    qt_t = []
    sp0 = [spsum.tile([rs, I], F32, tag=f"sp{kr}", name=f"sp{kr}")
           for kr, (r0, rs) in enumerate(KR_TILES)]

    def scores_step(b, sp, kf):
        for kr, (r0, rs) in enumerate(KR_TILES):
            nc.tensor.matmul(sp[kr][:], vfp_t[b][:, kf, r0:r0 + rs],
                             qt_t[kf][:], start=(kf == 0),
                             stop=(kf == FT - 1))

    def exp_es(sp):
        es_t = []
        for kr, (r0, rs) in enumerate(KR_TILES):
            es = esp.tile([rs, I], BF16, tag=f"es{kr}", name=f"es{kr}")
            with tc.high_priority():
                nc.scalar.activation(es[:], sp[kr][:],
                                     mybir.ActivationFunctionType.Exp,
                                     bias=ebias[0:rs, :], scale=1.0)
            es_t.append(es)
        return es_t

    with tc.tile_pool(name="qpsum", bufs=2, space=bass.MemorySpace.PSUM) as qpsum:
        for mf in range(FT):
            qp = qpsum.tile([128, I], F32, tag="qp")
            for k, (v0, vs) in enumerate(KV_TILES):
                nc.tensor.matmul(
                    qp[:], wa_t[k][:, I + mf * 128:I + (mf + 1) * 128],
                    wa_t[k][:, 0:I], start=(k == 0), stop=(k == 2))
            q = constp.tile([128, I], F16, tag=f"qt{mf}")
            nc.scalar.copy(q[:], qp[:])
            qt_t.append(q)
            if mf >= 2:
                scores_step(0, sp0, mf - 2)
        scores_step(0, sp0, FT - 2)
        scores_step(0, sp0, FT - 1)
    es0 = exp_es(sp0)
    opsum = ctx.enter_context(
        tc.tile_pool(name="opsum", bufs=5, space=bass.MemorySpace.PSUM))

    def scores_exp(b):
        es_t = []
        for kr, (r0, rs) in enumerate(KR_TILES):
            sp = spsum.tile([rs, I], F32, tag=f"sp{kr}", name=f"sp{kr}")
            for kf in range(FT):
                nc.tensor.matmul(sp[:], vfp_t[b][:, kf, r0:r0 + rs],
                                 qt_t[kf][:], start=(kf == 0),
                                 stop=(kf == FT - 1))
            es = esp.tile([rs, I], BF16, tag=f"es{kr}", name=f"es{kr}")
            with tc.high_priority():
                nc.scalar.activation(es[:], sp[:],
                                     mybir.ActivationFunctionType.Exp,
                                     bias=ebias[0:rs, :], scale=1.0)
            es_t.append(es)
        return es_t

    def sums_rcp_norm(b, es_t):
        sm = smpsum.tile([128, I], F32, tag="sm", name="sm")
        for kr, (r0, rs) in enumerate(KR_TILES):
            nc.tensor.matmul(sm[:], ones[0:rs, :], es_t[kr][:],
                             start=(kr == 0), stop=(kr == 1))
        rcpb = rcpp.tile([128, I], F32, tag="rcpb", name="rcpb")
        att_t = []
        with tc.high_priority():
            nc.vector.reciprocal(rcpb[:], sm[:])
            for kr, (r0, rs) in enumerate(KR_TILES):
                at = attp.tile([rs, I], F16, tag=f"at{kr}", name=f"at{kr}")
                nc.vector.tensor_tensor(at[:], es_t[kr][:], rcpb[0:rs, :],
                                        mybir.AluOpType.mult)
                att_t.append(at)
        return att_t

    def attend_part(b, att_t, otf, mfs, chunks):
        for mf in mfs:
            op_ = opsum.tile([128, 512], F32, tag="op", name="op")
            for kr, (r0, rs) in enumerate(KR_TILES):
                nc.tensor.matmul(
                    op_[:, 0:I],
                    vft_t[(b, kr)][:, mf * 128:(mf + 1) * 128],
                    att_t[kr][:], start=(kr == 0), stop=(kr == 1))
            dst = otf[:, mf, :]
            src = op_[:, 0:I]
            if mf % 2 == 0:
                nc.scalar.copy(dst, src)
            else:
                nc.vector.tensor_copy(dst, src)
            if mf in chunks:
                c0, cn, eng = chunks[mf]
                eng.dma_start(out[b, :, c0:c0 + cn, :],
                              otf[:, c0:c0 + cn, :])

    CH4 = {3: (0, 4), 7: (4, 4), 11: (8, 4), 15: (12, 4)}

    def attend(b, att_t, last=False):
        if last:
            chunks = {3: (0, 4), 7: (4, 4), 11: (8, 4),
                      13: (12, 2), 15: (14, 2)}
        else:
            chunks = CH4
        chunks = {mf: (c0, cn, nc.sync) for mf, (c0, cn) in chunks.items()}
        otf = outp.tile([128, FT, I], F16, tag="otf", name="otf")
        attend_part(b, att_t, otf, range(FT), chunks)

    for rep in range(reps):
        prev = None   # (b, att_t)
        for b in range(BL):
            es_t = es0 if (rep == 0 and b == 0) else scores_exp(b)
            if prev is None:
                att_t = sums_rcp_norm(b, es_t)
            elif b == BL - 1:
                # split the previous attend around this batch's sums so the
                # PE has work covering the rcp/normalize latency (there is no
                # scoresT(b+1) left to hide it behind)
                pb, patt = prev
                otf = outp.tile([128, FT, I], F16, tag="otf", name="otf")
                ch = {mf: (c0, cn, nc.sync) for mf, (c0, cn) in CH4.items()}
                attend_part(pb, patt, otf, range(0, 8), ch)
                att_t = sums_rcp_norm(b, es_t)
                attend_part(pb, patt, otf, range(8, FT), ch)
            else:
                attend(*prev)
                att_t = sums_rcp_norm(b, es_t)
            prev = (b, att_t)
        attend(prev[0], prev[1], last=(rep == reps - 1))


def _get_program(reps=1):
    key = ("nc", reps)
    if key in _CACHE:
        return _CACHE[key]
    nc = bacc.Bacc("TRN2", target_bir_lowering=False, debug=False,
                   num_devices=NCORES)
    waq_d = nc.dram_tensor("waq", [V, WQ], F16, kind="ExternalInput")
    vfp_d = nc.dram_tensor("vfp", [BL, 128, FT, R], F16, kind="ExternalInput")
    vft_d = nc.dram_tensor("vft", [BL, R, F], F16, kind="ExternalInput")
    out_d = nc.dram_tensor("out", [BL, 128, FT, I], F16,
                           kind="ExternalOutput")

    with tile.TileContext(nc) as tc, ExitStack() as ctx:
        _build_body(nc, tc, ctx, waq_d.ap(), vfp_d.ap(),
                    vft_d.ap(), out_d.ap(), reps)
    nc.compile()
    _CACHE[key] = nc
    return nc


def _prep_inputs(visual_features, v, W_alpha):
    vf = np.asarray(visual_features, dtype=np.float32)
    v = np.asarray(v, dtype=np.float32)
    W = np.asarray(W_alpha, dtype=np.float32)

    # packed [vT | W_alpha]: [V, I + F]
    waq16 = np.ascontiguousarray(
        np.concatenate([v.T, W], axis=1)).astype(np.float16)
    # [b, f, r] -> [b, p=128, t=16, r]  (f = t*128 + p)
    vfp16 = np.ascontiguousarray(
        vf.reshape(B, FT, 128, R).transpose(0, 2, 1, 3)).astype(np.float16)
    vft16 = np.ascontiguousarray(vf.transpose(0, 2, 1)).astype(np.float16)

    in_maps = []
    for c in range(NCORES):
        in_maps.append({
            "waq": waq16,
            "vfp": np.ascontiguousarray(vfp16[c * BL:(c + 1) * BL]),
            "vft": np.ascontiguousarray(vft16[c * BL:(c + 1) * BL]),
        })
    return in_maps


def kernel(visual_features, v, W_alpha):
    nc = _get_program()
    in_maps = _prep_inputs(visual_features, v, W_alpha)
    res = None
    for attempt in range(3):
        try:
            res = bass_utils.run_bass_kernel_spmd(
                nc, in_maps, core_ids=list(range(NCORES)))
            break
        except Exception:
            # transient NRT_EXEC_UNIT_UNRECOVERABLE wedges have been seen on
            # this fabric; a re-dispatch typically succeeds
            if attempt == 2:
                raise
    outs = [res.results[c]["out"] for c in range(NCORES)]
    buf = np.concatenate(outs, axis=0)          # [B, p=128, t=16, I]
    full = buf.transpose(0, 3, 2, 1).reshape(B, I, F)   # f = t*128 + p
    return np.ascontiguousarray(full).astype(np.float32)


# revision 40
# speedup vs baseline: 1.2775x; 1.0254x over previous
"""Trainium2 Bass kernel for attribute visual attention.

Computes, for each batch b:
    q      = v @ W_alpha                  # [i, f]
    scores = q @ vf[b]                    # [i, r]
    atten  = softmax(scores, axis=r)
    out[b] = atten @ vf[b].T              # [i, f]

Sharding: data-parallel over batch b across 8 NeuronCores (8 batches per
core); v / W_alpha replicated.

Design (v2 — transposed-scores pipeline):
- scores are computed TRANSPOSED: scoresT[r, i] = vf[b].T-contract via
  lhsT = vf[b] in its natural [f, r] layout, rhs = qT[f, i]. The softmax
  then needs no on-chip transposes at all: exp runs on scoresT directly
  (partition dim = r), and the attend matmul consumes esT[r, i] as the
  moving operand with lhsT = vfT[b] slices.
- No per-row max subtraction. scores*1.0 - 30.0 feeds Exp; results are
  stored in bf16 (range to 3e38 absorbs exp(~86) tails, and the +/-30
  shift cancels in the normalization). The softmax denominator comes from
  a ones-matmul that broadcasts the per-column sum to all 128 partitions
  in PSUM for free; normalization is a single DVE multiply on the small
  esT tiles (atten stored fp16 once normalized, values <= 1).
- PE work is software-pipelined as scoresT(k) -> attend(k-1) -> sums(k)
  so the tensor engine never waits on the ACT/DVE softmax chain.
- All inputs are DMA'd up front (everything fits in SBUF); inputs ride
  the HWDGE (sync) queue, outputs the SWDGE (gpsimd) queue.
- Attend outputs drain from PSUM via dual-bank copies (two 312-col tiles
  per instruction, alternating scalar/vector engines).
"""

import numpy as np
from contextlib import ExitStack

import concourse.bass as bass
import concourse.tile as tile
import concourse.bass_utils as bass_utils
from concourse import bacc, mybir

# Problem shapes (hardcoded per contest contract).
B, F, R, I, V = 64, 2048, 196, 312, 300
NCORES = 8
BL = B // NCORES          # 8 batches per core
FT = F // 128             # 16 f-tiles
KV_TILES = ((0, 128), (128, 128), (256, 44))    # v=300
KR_TILES = ((0, 128), (128, 68))                # r=196
EXP_SHIFT = -30.0

F16 = mybir.dt.float16
BF16 = mybir.dt.bfloat16
F32 = mybir.dt.float32

_CACHE = {}


WQ = I + F     # waq packed width: [vT | W_alpha]
WH = WQ // 2   # half-column DMA chunk


def _build_body(nc, tc, ctx, waq, vfp, vft, out, reps):
    constp = ctx.enter_context(tc.tile_pool(name="const", bufs=1))
    vfpp = ctx.enter_context(tc.tile_pool(name="vfp", bufs=1))
    vftp = ctx.enter_context(tc.tile_pool(name="vft", bufs=1))

    junk = constp.tile([128, 128], F16, tag="junk")
    ones = constp.tile([128, 128], BF16, tag="ones")
    ebias = constp.tile([128, 1], F32, tag="ebias")
    nc.vector.memset(junk[:], 0.03125)
    nc.vector.memset(ones[:], 1.0)
    nc.vector.memset(ebias[:], EXP_SHIFT)

    # ---- all input DMAs up front (everything is SBUF-resident).
    # Weights ride SP's HWDGE queue; the bulk vf loads ride the Pool/SWDGE
    # queue whose descriptor generator then has nothing else to do, so the
    # per-batch supply stays ahead of per-batch demand.  Outputs use SP. ----
    wa_t = {}
    with tc.high_priority():
        for h in range(2):
            for k, (v0, vs) in enumerate(KV_TILES):
                if h == 0:
                    w = constp.tile([vs, WQ], F16, tag=f"wa{k}", name=f"wa{k}")
                    wa_t[k] = w
                # three parallel HWDGE issue streams so the weight chunks are
                # not paced by a single SEQ's per-copy issue latency
                eng = (nc.sync, nc.scalar, nc.sync)[k]
                eng.dma_start(wa_t[k][:, h * WH:(h + 1) * WH],
                              waq[v0:v0 + vs, h * WH:(h + 1) * WH])
    vfp_t, vft_t = [], {}
    for b in range(BL):
        t = vfpp.tile([128, FT, R], F16, tag=f"vfp{b}")
        # early batches in t-chunks so the weight DMAs aren't stuck behind a
        # monolithic transfer and the fused batch-0 scores can start early
        nch = 4 if b == 0 else (2 if b == 1 else 1)
        step = FT // nch
        for c in range(nch):
            nc.gpsimd.dma_start(t[:, c * step:(c + 1) * step, :],
                                vfp[b, :, c * step:(c + 1) * step, :])
        vfp_t.append(t)
        for kr, (r0, rs) in enumerate(KR_TILES):
            v = vftp.tile([rs, F], F16, tag=f"vft{b}_{kr}")
            for hh in range(2):
                nc.gpsimd.dma_start(v[:, hh * 1024:(hh + 1) * 1024],
                                    vft[b, r0:r0 + rs, hh * 1024:(hh + 1) * 1024])
            vft_t[(b, kr)] = v

    # ---- PE warm-up while the first weight chunks land ----
    with tc.tile_pool(name="wupsum", bufs=1, space=bass.MemorySpace.PSUM) as wup:
        wu = wup.tile([128, 128], F32, tag="wu")
        for w in range(32):
            nc.tensor.matmul(wu[:], junk[:], junk[:],
                             start=(w == 0), stop=(w == 31))

    # ---- Phase 1: per-batch attention, PE-pipelined ----
    esp = ctx.enter_context(tc.tile_pool(name="es", bufs=6))
    attp = ctx.enter_context(tc.tile_pool(name="att", bufs=6))
    rcpp = ctx.enter_context(tc.tile_pool(name="rcp", bufs=3))
    outp = ctx.enter_context(tc.tile_pool(name="out", bufs=3))
    spsum = ctx.enter_context(
        tc.tile_pool(name="spsum", bufs=1, space=bass.MemorySpace.PSUM))
    smpsum = ctx.enter_context(
        tc.tile_pool(name="smpsum", bufs=1, space=bass.MemorySpace.PSUM))

    # ---- Phase 0 fused with batch 0's scores: the qT tiles are produced
    # two steps ahead of their use by scoresT(0), so batch 0's scores finish
    # ~right after the last q tile instead of a full scores-pass later. ----
    qt_t = []
    sp0 = [spsum.tile([rs, I], F32, tag=f"sp{kr}", name=f"sp{kr}")
           for kr, (r0, rs) in enumerate(KR_TILES)]

    def scores_step(b, sp, kf):
        for kr, (r0, rs) in enumerate(KR_TILES):
            nc.tensor.matmul(sp[kr][:], vfp_t[b][:, kf, r0:r0 + rs],
                             qt_t[kf][:], start=(kf == 0),
                             stop=(kf == FT - 1))

    def exp_es(sp):
        es_t = []
        for kr, (r0, rs) in enumerate(KR_TILES):
            es = esp.tile([rs, I], BF16, tag=f"es{kr}", name=f"es{kr}")
            with tc.high_priority():
                nc.scalar.activation(es[:], sp[kr][:],
                                     mybir.ActivationFunctionType.Exp,
                                     bias=ebias[0:rs, :], scale=1.0)
            es_t.append(es)
        return es_t

    with tc.tile_pool(name="qpsum", bufs=2, space=bass.MemorySpace.PSUM) as qpsum:
        for mf in range(FT):
            qp = qpsum.tile([128, I], F32, tag="qp")
            for k, (v0, vs) in enumerate(KV_TILES):
                nc.tensor.matmul(
                    qp[:], wa_t[k][:, I + mf * 128:I + (mf + 1) * 128],
                    wa_t[k][:, 0:I], start=(k == 0), stop=(k == 2))
            q = constp.tile([128, I], F16, tag=f"qt{mf}")
            nc.vector.tensor_copy(q[:], qp[:])
            qt_t.append(q)
            if mf >= 2:
                scores_step(0, sp0, mf - 2)
        scores_step(0, sp0, FT - 2)
        scores_step(0, sp0, FT - 1)
    es0 = exp_es(sp0)
    opsum = ctx.enter_context(
        tc.tile_pool(name="opsum", bufs=5, space=bass.MemorySpace.PSUM))

    def scores_exp(b, mid=None):
        es_t = []
        for kr, (r0, rs) in enumerate(KR_TILES):
            sp = spsum.tile([rs, I], F32, tag=f"sp{kr}", name=f"sp{kr}")
            for kf in range(FT):
                nc.tensor.matmul(sp[:], vfp_t[b][:, kf, r0:r0 + rs],
                                 qt_t[kf][:], start=(kf == 0),
                                 stop=(kf == FT - 1))
            es = esp.tile([rs, I], BF16, tag=f"es{kr}", name=f"es{kr}")
            with tc.high_priority():
                nc.scalar.activation(es[:], sp[:],
                                     mybir.ActivationFunctionType.Exp,
                                     bias=ebias[0:rs, :], scale=1.0)
            es_t.append(es)
            if kr == 0 and mid is not None:
                mid()
        return es_t

    def sums_rcp_norm(b, es_t):
        sm = smpsum.tile([128, I], F32, tag="sm", name="sm")
        for kr, (r0, rs) in enumerate(KR_TILES):
            nc.tensor.matmul(sm[:], ones[0:rs, :], es_t[kr][:],
                             start=(kr == 0), stop=(kr == 1))
        rcpb = rcpp.tile([128, I], F32, tag="rcpb", name="rcpb")
        att_t = []
        with tc.high_priority():
            nc.vector.reciprocal(rcpb[:], sm[:])
            for kr, (r0, rs) in enumerate(KR_TILES):
                at = attp.tile([rs, I], F16, tag=f"at{kr}", name=f"at{kr}")
                nc.vector.tensor_tensor(at[:], es_t[kr][:], rcpb[0:rs, :],
                                        mybir.AluOpType.mult)
                att_t.append(at)
        return att_t

    def attend_part(b, att_t, otf, mfs, chunks):
        for mf in mfs:
            op_ = opsum.tile([128, 512], F32, tag="op", name="op")
            for kr, (r0, rs) in enumerate(KR_TILES):
                nc.tensor.matmul(
                    op_[:, 0:I],
                    vft_t[(b, kr)][:, mf * 128:(mf + 1) * 128],
                    att_t[kr][:], start=(kr == 0), stop=(kr == 1))
            dst = otf[:, mf, :]
            src = op_[:, 0:I]
            if mf % 2 == 0:
                nc.scalar.copy(dst, src)
            else:
                nc.vector.tensor_copy(dst, src)
            if mf in chunks:
                c0, cn, eng = chunks[mf]
                eng.dma_start(out[b, :, c0:c0 + cn, :],
                              otf[:, c0:c0 + cn, :])

    CH4 = {3: (0, 4), 7: (4, 4), 11: (8, 4), 15: (12, 4)}

    def attend(b, att_t, last=False):
        if last:
            chunks = {3: (0, 4, nc.sync), 7: (4, 4, nc.sync),
                      11: (8, 4, nc.sync), 13: (12, 2, nc.sync),
                      15: (14, 2, nc.sync)}
        else:
            chunks = {mf: (c0, cn, nc.sync) for mf, (c0, cn) in CH4.items()}
        otf = outp.tile([128, FT, I], F16, tag="otf", name="otf")
        attend_part(b, att_t, otf, range(FT), chunks)

    for rep in range(reps):
        prev = None   # (b, att_t)
        for b in range(BL):
            if rep == 0 and b == 0:
                continue   # batch 0's scores were fused; sums deferred to b=1
            if rep == 0 and b == 1:
                # emit sums(0) between scoresT(1)'s two kr groups so the PE
                # has work while exp(0) finishes (nothing else fills it yet)
                hold = {}
                es_t = scores_exp(1, mid=lambda: hold.update(
                    a=sums_rcp_norm(0, es0)))
                prev = (0, hold["a"])
            else:
                es_t = scores_exp(b)
            if prev is None:
                att_t = sums_rcp_norm(b, es_t)
            elif b == BL - 1:
                # split the previous attend around this batch's sums so the
                # PE has work covering the rcp/normalize latency (there is no
                # scoresT(b+1) left to hide it behind)
                pb, patt = prev
                otf = outp.tile([128, FT, I], F16, tag="otf", name="otf")
                ch = {mf: (c0, cn, nc.sync) for mf, (c0, cn) in CH4.items()}
                attend_part(pb, patt, otf, range(0, 8), ch)
                att_t = sums_rcp_norm(b, es_t)
                attend_part(pb, patt, otf, range(8, FT), ch)
            else:
                attend(*prev)
                att_t = sums_rcp_norm(b, es_t)
            prev = (b, att_t)
        attend(prev[0], prev[1], last=(rep == reps - 1))


def _get_program(reps=1):
    key = ("nc", reps)
    if key in _CACHE:
        return _CACHE[key]
    nc = bacc.Bacc("TRN2", target_bir_lowering=False, debug=False,
                   num_devices=NCORES)
    waq_d = nc.dram_tensor("waq", [V, WQ], F16, kind="ExternalInput")
    vfp_d = nc.dram_tensor("vfp", [BL, 128, FT, R], F16, kind="ExternalInput")
    vft_d = nc.dram_tensor("vft", [BL, R, F], F16, kind="ExternalInput")
    out_d = nc.dram_tensor("out", [BL, 128, FT, I], F16,
                           kind="ExternalOutput")

    with tile.TileContext(nc) as tc, ExitStack() as ctx:
        _build_body(nc, tc, ctx, waq_d.ap(), vfp_d.ap(),
                    vft_d.ap(), out_d.ap(), reps)
    nc.compile()
    _CACHE[key] = nc
    return nc


def _prep_inputs(visual_features, v, W_alpha):
    vf = np.asarray(visual_features, dtype=np.float32)
    v = np.asarray(v, dtype=np.float32)
    W = np.asarray(W_alpha, dtype=np.float32)

    # packed [vT | W_alpha]: [V, I + F]
    waq16 = np.ascontiguousarray(
        np.concatenate([v.T, W], axis=1)).astype(np.float16)
    # [b, f, r] -> [b, p=128, t=16, r]  (f = t*128 + p)
    vfp16 = np.ascontiguousarray(
        vf.reshape(B, FT, 128, R).transpose(0, 2, 1, 3)).astype(np.float16)
    vft16 = np.ascontiguousarray(vf.transpose(0, 2, 1)).astype(np.float16)

    in_maps = []
    for c in range(NCORES):
        in_maps.append({
            "waq": waq16,
            "vfp": np.ascontiguousarray(vfp16[c * BL:(c + 1) * BL]),
            "vft": np.ascontiguousarray(vft16[c * BL:(c + 1) * BL]),
        })
    return in_maps


def kernel(visual_features, v, W_alpha):
    nc = _get_program()
    in_maps = _prep_inputs(visual_features, v, W_alpha)
    res = None
    for attempt in range(3):
        try:
            res = bass_utils.run_bass_kernel_spmd(
                nc, in_maps, core_ids=list(range(NCORES)))
            break
        except Exception:
            # transient NRT_EXEC_UNIT_UNRECOVERABLE wedges have been seen on
            # this fabric; a re-dispatch typically succeeds
            if attempt == 2:
                raise
    outs = [res.results[c]["out"] for c in range(NCORES)]
    buf = np.concatenate(outs, axis=0)          # [B, p=128, t=16, I]
    full = buf.transpose(0, 3, 2, 1).reshape(B, I, F)   # f = t*128 + p
    return np.ascontiguousarray(full).astype(np.float32)


# revision 41
# speedup vs baseline: 1.2786x; 1.0008x over previous
"""Trainium2 Bass kernel for attribute visual attention.

Computes, for each batch b:
    q      = v @ W_alpha                  # [i, f]
    scores = q @ vf[b]                    # [i, r]
    atten  = softmax(scores, axis=r)
    out[b] = atten @ vf[b].T              # [i, f]

Sharding: data-parallel over batch b across 8 NeuronCores (8 batches per
core); v / W_alpha replicated.

Design (v2 — transposed-scores pipeline):
- scores are computed TRANSPOSED: scoresT[r, i] = vf[b].T-contract via
  lhsT = vf[b] in its natural [f, r] layout, rhs = qT[f, i]. The softmax
  then needs no on-chip transposes at all: exp runs on scoresT directly
  (partition dim = r), and the attend matmul consumes esT[r, i] as the
  moving operand with lhsT = vfT[b] slices.
- No per-row max subtraction. scores*1.0 - 30.0 feeds Exp; results are
  stored in bf16 (range to 3e38 absorbs exp(~86) tails, and the +/-30
  shift cancels in the normalization). The softmax denominator comes from
  a ones-matmul that broadcasts the per-column sum to all 128 partitions
  in PSUM for free; normalization is a single DVE multiply on the small
  esT tiles (atten stored fp16 once normalized, values <= 1).
- PE work is software-pipelined as scoresT(k) -> attend(k-1) -> sums(k)
  so the tensor engine never waits on the ACT/DVE softmax chain.
- All inputs are DMA'd up front (everything fits in SBUF); inputs ride
  the HWDGE (sync) queue, outputs the SWDGE (gpsimd) queue.
- Attend outputs drain from PSUM via dual-bank copies (two 312-col tiles
  per instruction, alternating scalar/vector engines).
"""

import numpy as np
from contextlib import ExitStack

import concourse.bass as bass
import concourse.tile as tile
import concourse.bass_utils as bass_utils
from concourse import bacc, mybir

# Problem shapes (hardcoded per contest contract).
B, F, R, I, V = 64, 2048, 196, 312, 300
NCORES = 8
BL = B // NCORES          # 8 batches per core
FT = F // 128             # 16 f-tiles
KV_TILES = ((0, 128), (128, 128), (256, 44))    # v=300
KR_TILES = ((0, 128), (128, 68))                # r=196
EXP_SHIFT = -30.0

F16 = mybir.dt.float16
BF16 = mybir.dt.bfloat16
F32 = mybir.dt.float32

_CACHE = {}


WQ = I + F     # waq packed width: [vT | W_alpha]
WH = WQ // 2   # half-column DMA chunk


def _build_body(nc, tc, ctx, waq, vfp, vft, out, reps):
    constp = ctx.enter_context(tc.tile_pool(name="const", bufs=1))
    vfpp = ctx.enter_context(tc.tile_pool(name="vfp", bufs=1))
    vftp = ctx.enter_context(tc.tile_pool(name="vft", bufs=1))

    junk = constp.tile([128, 128], F16, tag="junk")
    ones = constp.tile([128, 128], BF16, tag="ones")
    ebias = constp.tile([128, 1], F32, tag="ebias")
    nc.vector.memset(junk[:], 0.03125)
    nc.vector.memset(ones[:], 1.0)
    nc.vector.memset(ebias[:], EXP_SHIFT)

    # ---- all input DMAs up front (everything is SBUF-resident).
    # Weights ride SP's HWDGE queue; the bulk vf loads ride the Pool/SWDGE
    # queue whose descriptor generator then has nothing else to do, so the
    # per-batch supply stays ahead of per-batch demand.  Outputs use SP. ----
    wa_t = {}
    with tc.high_priority():
        for h in range(2):
            for k, (v0, vs) in enumerate(KV_TILES):
                if h == 0:
                    w = constp.tile([vs, WQ], F16, tag=f"wa{k}", name=f"wa{k}")
                    wa_t[k] = w
                # three parallel HWDGE issue streams so the weight chunks are
                # not paced by a single SEQ's per-copy issue latency
                eng = (nc.sync, nc.scalar, nc.sync)[k]
                eng.dma_start(wa_t[k][:, h * WH:(h + 1) * WH],
                              waq[v0:v0 + vs, h * WH:(h + 1) * WH])
    vfp_t, vft_t = [], {}
    for b in range(BL):
        t = vfpp.tile([128, FT, R], F16, tag=f"vfp{b}")
        # early batches in t-chunks so the weight DMAs aren't stuck behind a
        # monolithic transfer and the fused batch-0 scores can start early
        nch = 4 if b == 0 else (2 if b == 1 else 1)
        step = FT // nch
        for c in range(nch):
            nc.gpsimd.dma_start(t[:, c * step:(c + 1) * step, :],
                                vfp[b, :, c * step:(c + 1) * step, :])
        vfp_t.append(t)
        for kr, (r0, rs) in enumerate(KR_TILES):
            v = vftp.tile([rs, F], F16, tag=f"vft{b}_{kr}")
            for hh in range(2):
                nc.gpsimd.dma_start(v[:, hh * 1024:(hh + 1) * 1024],
                                    vft[b, r0:r0 + rs, hh * 1024:(hh + 1) * 1024])
            vft_t[(b, kr)] = v

    # ---- PE warm-up while the first weight chunks land ----
    with tc.tile_pool(name="wupsum", bufs=1, space=bass.MemorySpace.PSUM) as wup:
        wu = wup.tile([128, 128], F32, tag="wu")
        for w in range(32):
            nc.tensor.matmul(wu[:], junk[:], junk[:],
                             start=(w == 0), stop=(w == 31))

    # ---- Phase 1: per-batch attention, PE-pipelined ----
    esp = ctx.enter_context(tc.tile_pool(name="es", bufs=6))
    attp = ctx.enter_context(tc.tile_pool(name="att", bufs=6))
    rcpp = ctx.enter_context(tc.tile_pool(name="rcp", bufs=3))
    outp = ctx.enter_context(tc.tile_pool(name="out", bufs=3))
    spsum = ctx.enter_context(
        tc.tile_pool(name="spsum", bufs=1, space=bass.MemorySpace.PSUM))
    smpsum = ctx.enter_context(
        tc.tile_pool(name="smpsum", bufs=1, space=bass.MemorySpace.PSUM))

    # ---- Phase 0 fused with batch 0's scores: the qT tiles are produced
    # two steps ahead of their use by scoresT(0), so batch 0's scores finish
    # ~right after the last q tile instead of a full scores-pass later. ----
    qt_t = []
    sp0 = [spsum.tile([rs, I], F32, tag=f"sp{kr}", name=f"sp{kr}")
           for kr, (r0, rs) in enumerate(KR_TILES)]

    def scores_step(b, sp, kf):
        for kr, (r0, rs) in enumerate(KR_TILES):
            nc.tensor.matmul(sp[kr][:], vfp_t[b][:, kf, r0:r0 + rs],
                             qt_t[kf][:], start=(kf == 0),
                             stop=(kf == FT - 1))

    def exp_es(sp):
        es_t = []
        for kr, (r0, rs) in enumerate(KR_TILES):
            es = esp.tile([rs, I], BF16, tag=f"es{kr}", name=f"es{kr}")
            with tc.high_priority():
                nc.scalar.activation(es[:], sp[kr][:],
                                     mybir.ActivationFunctionType.Exp,
                                     bias=ebias[0:rs, :], scale=1.0)
            es_t.append(es)
        return es_t

    with tc.tile_pool(name="qpsum", bufs=2, space=bass.MemorySpace.PSUM) as qpsum:
        for mf in range(FT):
            qp = qpsum.tile([128, I], F32, tag="qp")
            for k, (v0, vs) in enumerate(KV_TILES):
                nc.tensor.matmul(
                    qp[:], wa_t[k][:, I + mf * 128:I + (mf + 1) * 128],
                    wa_t[k][:, 0:I], start=(k == 0), stop=(k == 2))
            q = constp.tile([128, I], F16, tag=f"qt{mf}")
            nc.vector.tensor_copy(q[:], qp[:])
            qt_t.append(q)
            if mf >= 2:
                scores_step(0, sp0, mf - 2)
        scores_step(0, sp0, FT - 2)
        scores_step(0, sp0, FT - 1)
    es0 = exp_es(sp0)
    opsum = ctx.enter_context(
        tc.tile_pool(name="opsum", bufs=5, space=bass.MemorySpace.PSUM))

    def scores_exp(b, mid=None):
        es_t = []
        for kr, (r0, rs) in enumerate(KR_TILES):
            sp = spsum.tile([rs, I], F32, tag=f"sp{kr}", name=f"sp{kr}")
            for kf in range(FT):
                nc.tensor.matmul(sp[:], vfp_t[b][:, kf, r0:r0 + rs],
                                 qt_t[kf][:], start=(kf == 0),
                                 stop=(kf == FT - 1))
            es = esp.tile([rs, I], BF16, tag=f"es{kr}", name=f"es{kr}")
            with tc.high_priority():
                nc.scalar.activation(es[:], sp[:],
                                     mybir.ActivationFunctionType.Exp,
                                     bias=ebias[0:rs, :], scale=1.0)
            es_t.append(es)
            if kr == 0 and mid is not None:
                mid()
        return es_t

    def sums_rcp_norm(b, es_t):
        sm = smpsum.tile([128, I], F32, tag="sm", name="sm")
        for kr, (r0, rs) in enumerate(KR_TILES):
            nc.tensor.matmul(sm[:], ones[0:rs, :], es_t[kr][:],
                             start=(kr == 0), stop=(kr == 1))
        rcpb = rcpp.tile([128, I], F32, tag="rcpb", name="rcpb")
        att_t = []
        with tc.high_priority():
            nc.vector.reciprocal(rcpb[:], sm[:])
            for kr, (r0, rs) in enumerate(KR_TILES):
                at = attp.tile([rs, I], F16, tag=f"at{kr}", name=f"at{kr}")
                nc.vector.tensor_tensor(at[:], es_t[kr][:], rcpb[0:rs, :],
                                        mybir.AluOpType.mult)
                att_t.append(at)
        return att_t

    def attend_part(b, att_t, otf, mfs, chunks):
        for mf in mfs:
            op_ = opsum.tile([128, 512], F32, tag="op", name="op")
            for kr, (r0, rs) in enumerate(KR_TILES):
                nc.tensor.matmul(
                    op_[:, 0:I],
                    vft_t[(b, kr)][:, mf * 128:(mf + 1) * 128],
                    att_t[kr][:], start=(kr == 0), stop=(kr == 1))
            dst = otf[:, mf, :]
            src = op_[:, 0:I]
            if mf % 2 == 0:
                nc.scalar.copy(dst, src)
            else:
                nc.vector.tensor_copy(dst, src)
            if mf in chunks:
                c0, cn, eng = chunks[mf]
                eng.dma_start(out[b, :, c0:c0 + cn, :],
                              otf[:, c0:c0 + cn, :])

    CH4 = {3: (0, 4), 7: (4, 4), 11: (8, 4), 15: (12, 4)}

    def attend(b, att_t, last=False):
        if last:
            chunks = {3: (0, 4, nc.sync), 7: (4, 4, nc.sync),
                      11: (8, 4, nc.sync), 13: (12, 2, nc.scalar),
                      15: (14, 2, nc.sync)}
        else:
            chunks = {mf: (c0, cn, nc.sync) for mf, (c0, cn) in CH4.items()}
        otf = outp.tile([128, FT, I], F16, tag="otf", name="otf")
        attend_part(b, att_t, otf, range(FT), chunks)

    for rep in range(reps):
        prev = None   # (b, att_t)
        for b in range(BL):
            if rep == 0 and b == 0:
                continue   # batch 0's scores were fused; sums deferred to b=1
            if rep == 0 and b == 1:
                # emit sums(0) between scoresT(1)'s two kr groups so the PE
                # has work while exp(0) finishes (nothing else fills it yet)
                hold = {}
                es_t = scores_exp(1, mid=lambda: hold.update(
                    a=sums_rcp_norm(0, es0)))
                prev = (0, hold["a"])
            else:
                es_t = scores_exp(b)
            if prev is None:
                att_t = sums_rcp_norm(b, es_t)
            elif b == BL - 1:
                # split the previous attend around this batch's sums so the
                # PE has work covering the rcp/normalize latency (there is no
                # scoresT(b+1) left to hide it behind)
                pb, patt = prev
                otf = outp.tile([128, FT, I], F16, tag="otf", name="otf")
                ch = {mf: (c0, cn, nc.sync) for mf, (c0, cn) in CH4.items()}
                attend_part(pb, patt, otf, range(0, 8), ch)
                att_t = sums_rcp_norm(b, es_t)
                attend_part(pb, patt, otf, range(8, FT), ch)
            else:
                attend(*prev)
                att_t = sums_rcp_norm(b, es_t)
            prev = (b, att_t)
        attend(prev[0], prev[1], last=(rep == reps - 1))


def _get_program(reps=1):
    key = ("nc", reps)
    if key in _CACHE:
        return _CACHE[key]
    nc = bacc.Bacc("TRN2", target_bir_lowering=False, debug=False,
                   num_devices=NCORES)
    waq_d = nc.dram_tensor("waq", [V, WQ], F16, kind="ExternalInput")
    vfp_d = nc.dram_tensor("vfp", [BL, 128, FT, R], F16, kind="ExternalInput")
    vft_d = nc.dram_tensor("vft", [BL, R, F], F16, kind="ExternalInput")
    out_d = nc.dram_tensor("out", [BL, 128, FT, I], F16,
                           kind="ExternalOutput")

    with tile.TileContext(nc) as tc, ExitStack() as ctx:
        _build_body(nc, tc, ctx, waq_d.ap(), vfp_d.ap(),
                    vft_d.ap(), out_d.ap(), reps)
    nc.compile()
    _CACHE[key] = nc
    return nc


def _prep_inputs(visual_features, v, W_alpha):
    vf = np.asarray(visual_features, dtype=np.float32)
    v = np.asarray(v, dtype=np.float32)
    W = np.asarray(W_alpha, dtype=np.float32)

    # packed [vT | W_alpha]: [V, I + F]
    waq16 = np.ascontiguousarray(
        np.concatenate([v.T, W], axis=1)).astype(np.float16)
    # [b, f, r] -> [b, p=128, t=16, r]  (f = t*128 + p)
    vfp16 = np.ascontiguousarray(
        vf.reshape(B, FT, 128, R).transpose(0, 2, 1, 3)).astype(np.float16)
    vft16 = np.ascontiguousarray(vf.transpose(0, 2, 1)).astype(np.float16)

    in_maps = []
    for c in range(NCORES):
        in_maps.append({
            "waq": waq16,
            "vfp": np.ascontiguousarray(vfp16[c * BL:(c + 1) * BL]),
            "vft": np.ascontiguousarray(vft16[c * BL:(c + 1) * BL]),
        })
    return in_maps


def kernel(visual_features, v, W_alpha):
    nc = _get_program()
    in_maps = _prep_inputs(visual_features, v, W_alpha)
    res = None
    for attempt in range(3):
        try:
            res = bass_utils.run_bass_kernel_spmd(
                nc, in_maps, core_ids=list(range(NCORES)))
            break
        except Exception:
            # transient NRT_EXEC_UNIT_UNRECOVERABLE wedges have been seen on
            # this fabric; a re-dispatch typically succeeds
            if attempt == 2:
                raise
    outs = [res.results[c]["out"] for c in range(NCORES)]
    buf = np.concatenate(outs, axis=0)          # [B, p=128, t=16, I]
    full = buf.transpose(0, 3, 2, 1).reshape(B, I, F)   # f = t*128 + p
    return np.ascontiguousarray(full).astype(np.float32)
